# revision 31
# baseline (speedup 1.0000x reference)
"""Fused multi-head causal+padding attention for Trainium2 (Bass/Tile).

Problem: nn_Attention (B=8, T=1024, C=512, H=8, D=64, TT=4), f32.
Sharding: data-parallel over batch B across 8 NeuronCores (1 batch elem/core).

v2 design (all-bf16 matmuls, causal block-skip + padded-row fix-up):
  mask[q,k] = (k <= q) | pad[q]  -> 90% of rows are causal, 10% attend to all k.
  - QKV: qk^T = W_qk^T @ x^T (heads in [d,t] layout), v = x @ W_v ([t,d]).
    x arrives pre-transposed from host (both x and x^T in bf16): no on-chip
    transposes. DMAs are split per-contraction-chunk so the first matmuls
    start after ~1/4 of the weight load.
  - Causal pass (per head, k-tile strips): S^T[k, q>=128*kt] only (skips
    fully-masked above-diagonal blocks: 36/64 remain), one exp per strip
    (late small strips paired to amortize ACT PSUM-access init), bf16
    multiplicative mask on the diagonal 128-block only, AV accumulates
    y[65, q] with an appended ones column as the softmax denominator.
    Heads are software-pipelined: QK strips of head h+1 are emitted before
    the AVs of head h so PE fills the exp latency.
  - Pad pass: padded-row queries are gathered via a host-built one-hot
    matrix G (gather = matmul), projected to q_pad, attended over exactly
    the k-tiles the causal pass skipped, and the raw numerator/denominator
    contributions are scattered back into each head's PSUM accumulator
    with G^T (scatter = matmul, accumulate). Slots are grouped by query
    tile (same slot->tile profile on every core, dummy zero-columns pad
    the groups), so k-tile kt only processes the slot prefix cnt[kt] and
    no pad-side masking is needed at all.
  - Normalize via reciprocal + partition_broadcast, then out^T = W_p^T y^T
    (+bias) stored transposed; host untransposes.

HW gotchas encoded here: 64-row matmuls must keep a fixed lhsT partition
base between nearby instructions (PE row-quadrant switching aborts the
NEFF), PSUM matmul start=True zeroes the whole 2KB bank (one accumulation
group per bank; K=1 zeros-matmul opens shared banks), GPSIMD cannot touch
PSUM. All matmuls bf16 (1 cycle/row at any N).
"""

import numpy as np
import ml_dtypes
from contextlib import ExitStack

B, T, C, H, TT = 8, 1024, 512, 8, 4
D = C // H
NCORES = 8
TK = T // 128   # 8 t-tiles
CK = C // 128   # 4 c-tiles
J = 128         # gathered padded-row slot capacity (profile sum <= 128)

_CACHE = {}
_CNT = {"profile": None}   # slot-prefix profile, set by make_in_maps

# exp strip pairing: kt -> (tile key, column offset inside the tile)
_PAIR = {0: (0, 0), 1: (1, 0), 2: (2, 0), 3: (3, 0),
         4: (4, 0), 5: (4, 512), 6: (6, 0), 7: (6, 256)}


def _chunks(w, step=512):
    off = 0
    while off < w:
        n = min(step, w - off)
        yield off, n
        off += n


def _build_nc(reps=1, cnt=None):
    import concourse.mybir as mybir
    import concourse.tile as tile
    from concourse import bacc
    from concourse.bass import ts

    dt = mybir.dt
    F32, BF16 = dt.float32, dt.bfloat16
    AF = mybir.ActivationFunctionType
    if cnt is None:
        cnt = (0,) + (J,) * (TK - 1)

    nc = bacc.Bacc("TRN2", target_bir_lowering=False, debug=False,
                   num_devices=NCORES)

    xT_d = nc.dram_tensor("xT", [128, CK, T], BF16, kind="ExternalInput").ap()
    xb_d = nc.dram_tensor("xb", [128, TK, C], BF16, kind="ExternalInput").ap()
    wqk_d = nc.dram_tensor("wqk", [128, CK, 2 * C], BF16, kind="ExternalInput").ap()
    wv_d = nc.dram_tensor("wv", [128, CK, C], BF16, kind="ExternalInput").ap()
    wp_d = nc.dram_tensor("wp", [128, CK, C], BF16, kind="ExternalInput").ap()
    bqk_d = nc.dram_tensor("bqk", [128, 2 * C // 128], F32, kind="ExternalInput").ap()
    beff_d = nc.dram_tensor("beff", [128, CK], F32, kind="ExternalInput").ap()
    dmask_d = nc.dram_tensor("dmask", [128, TK, 128], BF16, kind="ExternalInput").ap()
    G_d = nc.dram_tensor("G", [128, TK, J], BF16, kind="ExternalInput").ap()
    GT_d = nc.dram_tensor("GT", [J, T], BF16, kind="ExternalInput").ap()
    out_d = nc.dram_tensor("out", [128, CK, T], F32, kind="ExternalOutput").ap()

    with tile.TileContext(nc) as tc, ExitStack() as ctx:
        consts = ctx.enter_context(tc.tile_pool(name="consts", bufs=1))

        # dummy exp pulls the ACT exp-table load into the DMA-bound front
        warm = consts.tile([1, 128], F32)
        nc.gpsimd.memset(warm, 0.0)
        nc.scalar.activation(warm, warm, AF.Exp)

        # per-chunk loads so the first qk matmuls start after ~1/4 of the
        # weight+activation load; wp is only needed by the final proj.
        xT_s = consts.tile([128, CK, T], BF16)
        wqk_s = consts.tile([128, CK, 2 * C], BF16)
        for j in range(CK):
            nc.sync.dma_start(out=xT_s[:, j, :], in_=xT_d[:, j, :])
            nc.scalar.dma_start(out=wqk_s[:, j, :], in_=wqk_d[:, j, :])
        xb_s = consts.tile([128, TK, C], BF16)
        nc.sync.dma_start(out=xb_s, in_=xb_d)
        G_s = consts.tile([128, TK, J], BF16)
        nc.gpsimd.dma_start(out=G_s, in_=G_d)
        wv_s = consts.tile([128, CK, C], BF16)
        nc.scalar.dma_start(out=wv_s, in_=wv_d)
        bqk_s = consts.tile([128, 2 * C // 128], F32)
        nc.gpsimd.dma_start(out=bqk_s, in_=bqk_d)
        dmask_s = consts.tile([128, TK, 128], BF16)
        nc.gpsimd.dma_start(out=dmask_s, in_=dmask_d)
        GT_s = consts.tile([J, T], BF16)
        nc.gpsimd.dma_start(out=GT_s, in_=GT_d)
        beff_s = consts.tile([128, CK], F32)
        nc.gpsimd.dma_start(out=beff_s, in_=beff_d)
        wp_s = consts.tile([128, CK, C], BF16)
        nc.scalar.dma_start(out=wp_s, in_=wp_d)

        zrow = consts.tile([1, 512], BF16)
        nc.gpsimd.memset(zrow, 0.0)
        zcol = consts.tile([1, 128], BF16)
        nc.gpsimd.memset(zcol, 0.0)

        qkT = consts.tile([128, 2 * C // 128, T], BF16)   # tiles 0-3 q, 4-7 k
        vaug = consts.tile([128, TK, H, D + 1], BF16)
        xgT = consts.tile([128, CK, J], BF16)
        qpT = consts.tile([128, CK, J], BF16)
        ypad = consts.tile([J, 2, 4 * (D + 1)], BF16)     # raw pad num/den
        yT = consts.tile([128, CK, T], BF16)

        nc.gpsimd.memset(vaug[:, :, :, D:D + 1], 1.0)

        def run_body():
            body(nc, tc, ts, F32, BF16, AF, cnt,
                 xb_s, G_s, xT_s, wqk_s, wv_s, wp_s, bqk_s, beff_s,
                 dmask_s, GT_s, zrow, zcol, qkT, vaug, xgT, qpT, ypad, yT,
                 out_d)

        if reps == 1:
            run_body()
        else:
            with tc.For_i(0, reps, 1):
                run_body()

    nc.compile()
    return nc


def body(nc, tc, ts, F32, BF16, AF, cnt,
         xb_s, G_s, xT_s, wqk_s, wv_s, wp_s, bqk_s, beff_s,
         dmask_s, GT_s, zrow, zcol, qkT, vaug, xgT, qpT, ypad, yT, out_d):
    # ---- QKV projections + pad-row gather/QK/exp/AV, interleaved ----
    with tc.tile_pool(name="pqk", bufs=3, space="PSUM") as pqk, \
         tc.tile_pool(name="etpad", bufs=14) as etpad:

        def qk_tile(i):
            for n in range(T // 512):
                ps = pqk.tile([128, 512], F32)
                for j in range(CK):
                    nc.tensor.matmul(ps, wqk_s[:, j, ts(i, 128)],
                                     xT_s[:, j, ts(n, 512)],
                                     start=(j == 0), stop=(j == CK - 1))
                nc.vector.tensor_scalar_add(qkT[:, i, ts(n, 512)], ps,
                                            bqk_s[:, i:i + 1])

        def v_tile(i):
            ps = pqk.tile([128, 512], F32)
            for j in range(CK):
                nc.tensor.matmul(ps, xT_s[:, j, ts(i, 128)], wv_s[:, j, :],
                                 start=(j == 0), stop=(j == CK - 1))
            nc.scalar.activation(
                vaug[:, i, :, 0:D], ps.rearrange("p (h d) -> p h d", h=H),
                AF.Copy)

        # k head-tiles first: the pad pass needs them and PE can start as
        # soon as the first wqk/xT chunks land
        for i in range(4, 8):
            qk_tile(i)

        # gather padded-row x columns (xgT = x^T @ G) and project to q_pad
        # (pgath closes before pspad/ypadp open: PSUM is fully budgeted)
        with tc.tile_pool(name="pgath", bufs=2, space="PSUM") as pg:
            for ct in range(CK):
                ps = pg.tile([128, J], F32)
                for tcp in range(TK):
                    nc.tensor.matmul(ps, xb_s[:, tcp, ts(ct, 128)],
                                     G_s[:, tcp, :],
                                     start=(tcp == 0), stop=(tcp == TK - 1))
                nc.vector.tensor_copy(xgT[:, ct, :], ps)
            for pt in range(CK):
                ps = pg.tile([128, J], F32)
                for j in range(CK):
                    nc.tensor.matmul(ps, wqk_s[:, j, ts(pt, 128)],
                                     xgT[:, j, :],
                                     start=(j == 0), stop=(j == CK - 1))
                nc.vector.tensor_scalar_add(qpT[:, pt, :], ps,
                                            bqk_s[:, pt:pt + 1])

        # pad strips grouped by head parity: 64-row matmuls must keep a
        # fixed lhsT partition-base within a strip (switching the PE
        # row-quadrant between nearby 64-row matmuls aborts the NEFF), so
        # group pg covers heads h = 2*hh + pg, all at base pg*64. Strip kt
        # only computes the slot prefix [0:cnt[kt]]. q/v tiles interleave
        # as PE filler while ACT chews the pad exps.
        eps = {}
        filler = [(qk_tile, 0), (qk_tile, 1), (v_tile, 0), (v_tile, 1),
                  (qk_tile, 2), (qk_tile, 3), (v_tile, 2), (v_tile, 3),
                  (v_tile, 4), (v_tile, 5), (v_tile, 6), (v_tile, 7)]
        fi = 0
        pads = ExitStack()
        pspad = pads.enter_context(
            tc.tile_pool(name="pspad", bufs=2, space="PSUM"))
        ypadp = pads.enter_context(
            tc.tile_pool(name="ypadp", bufs=1, space="PSUM"))
        for pg_ in range(2):
            po = pg_ * 64
            for kt in range(1, TK):
                ck = cnt[kt]
                if ck:
                    sp = pspad.tile([128, 4, J], F32)
                    for hh in range(4):
                        nc.tensor.matmul(sp[:, hh, 0:ck],
                                         qkT[po:po + D, 4 + hh, ts(kt, 128)],
                                         qpT[po:po + D, hh, 0:ck],
                                         start=(hh == 0), stop=(hh == 3),
                                         skip_group_check=True)
                    ep = etpad.tile([128, 4, J], BF16)
                    nc.scalar.activation(ep[:, :, 0:ck], sp[:, :, 0:ck],
                                         AF.Exp)
                    eps[(kt, pg_)] = ep
                if fi < len(filler):
                    f, arg = filler[fi]
                    f(arg)
                    fi += 1
        while fi < len(filler):
            f, arg = filler[fi]
            f(arg)
            fi += 1

        # pad AV: one K=1 zeros-matmul opens each half's 2KB zero-region
        # exactly once (start=True zeroes the whole bank), then everything
        # accumulates; rows >= cnt[kt] stay zero and scatter ignores them
        ypp = ypadp.tile([J, 2, 512], F32)
        for pg_ in range(2):
            nc.tensor.matmul(ypp[:, pg_, :], zcol, zrow,
                             start=True, stop=False, skip_group_check=True)
        for pg_ in range(2):
            for kt in range(1, TK):
                ck = cnt[kt]
                if not ck:
                    continue
                ep = eps[(kt, pg_)]
                for hh in range(4):
                    h = 2 * hh + pg_
                    nc.tensor.matmul(ypp[0:ck, pg_, ts(hh, D + 1)],
                                     ep[:, hh, 0:ck], vaug[:, kt, h, :],
                                     start=False,
                                     stop=(kt == TK - 1 and hh == 3),
                                     skip_group_check=True)

        nc.vector.tensor_copy(ypad[:, 0, :], ypp[:, 0, 0:4 * (D + 1)])
        nc.vector.tensor_copy(ypad[:, 1, :], ypp[:, 1, 0:4 * (D + 1)])
        pads.close()

    # ---- causal attention, head-outer, software-pipelined across heads ----
    with tc.tile_pool(name="ps_s", bufs=2, space="PSUM") as ps_s, \
         tc.tile_pool(name="ps_y", bufs=2, space="PSUM") as ps_y, \
         tc.tile_pool(name="expp", bufs=14) as expp, \
         tc.tile_pool(name="rp", bufs=2) as rp, \
         tc.tile_pool(name="rbp", bufs=2) as rbp:
        ets = {}

        def emit_qk(h):
            # k-tile strips; the late small strips share one PSUM tile and
            # one exp (single accumulation group per bank; mid-bank output
            # offsets are fine)
            hp, po = h // 2, (h % 2) * 64
            sps = {}
            for kt in range(TK):
                key, poff = _PAIR[kt]
                if kt == key:
                    sp = ps_s.tile([128, T], F32)
                    sps[key] = sp
                else:
                    sp = sps[key]
                q0 = 128 * kt
                w = T - q0
                for off, n in _chunks(w):
                    o = poff + off
                    nc.tensor.matmul(sp[:, o:o + n],
                                     qkT[po:po + D, 4 + hp, ts(kt, 128)],
                                     qkT[po:po + D, hp, q0 + off:q0 + off + n],
                                     start=(o % 512 == 0), stop=(kt == key + 1
                                                                 or kt <= 3),
                                     skip_group_check=True)
                if kt == key + 1 or kt <= 3:
                    et = expp.tile([128, T], BF16)
                    wtot = poff + w
                    nc.scalar.activation(et[:, 0:wtot], sp[:, 0:wtot], AF.Exp)
                    ets[(h, key)] = et
            for kt in range(TK):
                key, poff = _PAIR[kt]
                nc.vector.tensor_mul(ets[(h, key)][:, poff:poff + 128],
                                     ets[(h, key)][:, poff:poff + 128],
                                     dmask_s[:, kt, :])

        def emit_av(h):
            hp, po = h // 2, (h % 2) * 64
            half, hh = h % 2, h // 2
            yp = ps_y.tile([D + 1, T], F32)
            for kt in range(TK):
                key, poff = _PAIR[kt]
                et = ets[(h, key)]
                q0 = 128 * kt
                a = q0
                for bnd in (512, T):
                    if a < bnd:
                        n = bnd - a
                        nc.tensor.matmul(
                            yp[:, a:a + n], vaug[:, kt, h, :],
                            et[:, poff + a - q0:poff + a - q0 + n],
                            start=(kt == 0), stop=False,
                            skip_group_check=True)
                        a = bnd
            # scatter pad-row num/den contributions into this head's PSUM
            for off, n in _chunks(T):
                nc.tensor.matmul(yp[:, off:off + n],
                                 ypad[:, half, ts(hh, D + 1)],
                                 GT_s[:, off:off + n],
                                 start=False, stop=True,
                                 skip_group_check=True)
            rec = rp.tile([1, T], F32)
            nc.vector.reciprocal(rec, yp[D:D + 1, :])
            rb = rbp.tile([D, T], F32)
            nc.gpsimd.partition_broadcast(rb, rec)
            nc.vector.tensor_mul(yT[po:po + D, hp, :], yp[0:D, :], rb)

        for h in range(H):
            emit_qk(h)
            if h >= 1:
                emit_av(h - 1)
        emit_av(H - 1)

    # ---- out^T = W_p^T y^T + b_eff, stored transposed ----
    with tc.tile_pool(name="pp", bufs=2, space="PSUM") as pp, \
         tc.tile_pool(name="outst", bufs=3) as outst:
        for qt in range(TK):
            op = pp.tile([128, CK, 128], F32)
            for ct in range(CK):
                for cin in range(CK):
                    nc.tensor.matmul(op[:, ct, :],
                                     wp_s[:, cin, ts(ct, 128)],
                                     yT[:, cin, ts(qt, 128)],
                                     start=(ct == 0 and cin == 0),
                                     stop=(ct == CK - 1 and cin == CK - 1),
                                     skip_group_check=True)
            ot = outst.tile([128, CK, 128], F32)
            for ct in range(CK):
                if ct % 2 == 0:
                    nc.scalar.activation(ot[:, ct, :], op[:, ct, :],
                                         AF.Identity,
                                         bias=beff_s[:, ct:ct + 1])
                else:
                    nc.vector.tensor_scalar_add(ot[:, ct, :], op[:, ct, :],
                                                beff_s[:, ct:ct + 1])
            nc.sync.dma_start(out=out_d[:, :, ts(qt, 128)], in_=ot)


def get_nc(reps=1):
    cnt = _CNT["profile"]
    key = ("nc", reps, cnt)
    if key not in _CACHE:
        _CACHE[key] = _build_nc(reps, cnt)
    return _CACHE[key]


def _slot_layout(padding_mask):
    """Group padded-row slots by query tile with capacities shared across
    all TT groups so one NEFF serves every core; returns the layout."""
    counts = np.zeros((TT, TK), np.int64)
    for tt in range(TT):
        qt = np.where(padding_mask[tt])[0] // 128
        for g in range(TK):
            counts[tt, g] = (qt == g).sum()
    caps = counts.max(0)
    assert caps.sum() <= J, f"slot profile {caps.sum()} exceeds J={J}"
    slot0 = np.concatenate([[0], np.cumsum(caps)])
    cnt = tuple(int(slot0[kt]) for kt in range(TK))  # slots with group < kt
    return slot0, cnt


def make_in_maps(x, padding_mask, W_qkv, b_qkv, W_proj, b_proj):
    BF = ml_dtypes.bfloat16
    x = np.asarray(x, np.float32)
    padding_mask = np.asarray(padding_mask, bool)
    W_qkv = np.asarray(W_qkv, np.float32)
    b_qkv = np.asarray(b_qkv, np.float32)
    W_proj = np.asarray(W_proj, np.float32)
    b_proj = np.asarray(b_proj, np.float32)

    scale = np.float32(1.0 / np.sqrt(D))
    wqk = np.concatenate([W_qkv[:, :C] * scale, W_qkv[:, C:2 * C]], axis=1)
    wqk = np.ascontiguousarray(
        wqk.reshape(CK, 128, 2 * C).transpose(1, 0, 2)).astype(BF)
    wv = np.ascontiguousarray(
        W_qkv[:, 2 * C:].reshape(CK, 128, C).transpose(1, 0, 2)).astype(BF)
    wp = np.ascontiguousarray(
        W_proj.reshape(CK, 128, C).transpose(1, 0, 2)).astype(BF)
    bqk = np.concatenate([b_qkv[:C] * scale, b_qkv[C:2 * C]])
    bqk = np.ascontiguousarray(bqk.reshape(-1, 128).T).astype(np.float32)
    beff = (b_qkv[2 * C:] @ W_proj + b_proj)
    beff = np.ascontiguousarray(beff.reshape(CK, 128).T).astype(np.float32)

    slot0, cnt = _slot_layout(padding_mask)
    _CNT["profile"] = cnt

    pp = np.arange(128)
    in_maps = []
    for b in range(B):
        tt = b % TT
        pad = padding_mask[tt]
        idx = np.where(pad)[0]
        # slot assignment: query-tile-g rows occupy slots [slot0[g], ...)
        slots = np.empty(len(idx), np.int64)
        for g in range(TK):
            sel = (idx // 128) == g
            slots[sel] = slot0[g] + np.arange(sel.sum())

        G = np.zeros((T, J), np.float32)
        G[idx, slots] = 1.0
        G = np.ascontiguousarray(
            G.reshape(TK, 128, J).transpose(1, 0, 2)).astype(BF)
        GT = np.zeros((J, T), np.float32)
        GT[slots, idx] = 1.0
        GT = GT.astype(BF)
        # dmask[p, kt, q] = (p <= q) | pad[128*kt + q]
        dmask = (pp[:, None, None] <= pp[None, None, :]) | \
            pad.reshape(TK, 128)[None, :, :]
        dmask = np.ascontiguousarray(dmask).astype(BF)

        xb_full = x[b]
        xT = np.ascontiguousarray(
            xb_full.T.reshape(CK, 128, T).transpose(1, 0, 2)).astype(BF)
        xb = np.ascontiguousarray(
            xb_full.reshape(TK, 128, C).transpose(1, 0, 2)).astype(BF)

        in_maps.append({
            "xT": xT, "xb": xb, "wqk": wqk, "wv": wv, "wp": wp,
            "bqk": bqk, "beff": beff, "dmask": dmask,
            "G": G, "GT": GT,
        })
    return in_maps


def kernel(x, padding_mask, W_qkv, b_qkv, W_proj, b_proj):
    from concourse.bass_utils import run_bass_kernel_spmd

    in_maps = make_in_maps(x, padding_mask, W_qkv, b_qkv, W_proj, b_proj)
    nc = get_nc()
    res = run_bass_kernel_spmd(nc, in_maps, list(range(NCORES)))
    outs = []
    for b in range(B):
        a = res.results[b]["out"]          # [128, CK, T] = out^T tiled
        outT = a.transpose(1, 0, 2).reshape(C, T)
        outs.append(outT.T)
    return np.ascontiguousarray(np.stack(outs)).astype(np.float32)


# revision 33
# speedup vs baseline: 1.0264x; 1.0264x over previous
"""Fused multi-head causal+padding attention for Trainium2 (Bass/Tile).

Problem: nn_Attention (B=8, T=1024, C=512, H=8, D=64, TT=4), f32.
Sharding: data-parallel over batch B across 8 NeuronCores (1 batch elem/core).

v2 design (all-bf16 matmuls, causal block-skip + padded-row fix-up):
  mask[q,k] = (k <= q) | pad[q]  -> 90% of rows are causal, 10% attend to all k.
  - QKV: qk^T = W_qk^T @ x^T (heads in [d,t] layout), v = x @ W_v ([t,d]).
    x arrives pre-transposed from host (both x and x^T in bf16): no on-chip
    transposes. DMAs are split per-contraction-chunk so the first matmuls
    start after ~1/4 of the weight load.
  - Causal pass (per head, k-tile strips): S^T[k, q>=128*kt] only (skips
    fully-masked above-diagonal blocks: 36/64 remain), one exp per strip
    (late small strips paired to amortize ACT PSUM-access init), bf16
    multiplicative mask on the diagonal 128-block only, AV accumulates
    y[65, q] with an appended ones column as the softmax denominator.
    Heads are software-pipelined: QK strips of head h+1 are emitted before
    the AVs of head h so PE fills the exp latency.
  - Pad pass: padded-row queries are gathered via a host-built one-hot
    matrix G (gather = matmul), projected to q_pad, attended over exactly
    the k-tiles the causal pass skipped, and the raw numerator/denominator
    contributions are scattered back into each head's PSUM accumulator
    with G^T (scatter = matmul, accumulate). Slots are grouped by query
    tile (same slot->tile profile on every core, dummy zero-columns pad
    the groups), so k-tile kt only processes the slot prefix cnt[kt] and
    no pad-side masking is needed at all.
  - Normalize via reciprocal + partition_broadcast, then out^T = W_p^T y^T
    (+bias) stored transposed; host untransposes.

HW gotchas encoded here: 64-row matmuls must keep a fixed lhsT partition
base between nearby instructions (PE row-quadrant switching aborts the
NEFF), PSUM matmul start=True zeroes the whole 2KB bank (one accumulation
group per bank; K=1 zeros-matmul opens shared banks), GPSIMD cannot touch
PSUM. All matmuls bf16 (1 cycle/row at any N).
"""

import numpy as np
import ml_dtypes
from contextlib import ExitStack

B, T, C, H, TT = 8, 1024, 512, 8, 4
D = C // H
NCORES = 8
TK = T // 128   # 8 t-tiles
CK = C // 128   # 4 c-tiles
J = 128         # gathered padded-row slot capacity (profile sum <= 128)

_CACHE = {}
_CNT = {"profile": None}   # slot-prefix profile, set by make_in_maps

# exp strip pairing: kt -> (tile key, column offset inside the tile)
_PAIR = {0: (0, 0), 1: (1, 0), 2: (2, 0), 3: (3, 0),
         4: (4, 0), 5: (4, 512), 6: (6, 0), 7: (6, 256)}


def _chunks(w, step=512):
    off = 0
    while off < w:
        n = min(step, w - off)
        yield off, n
        off += n


def _build_nc(reps=1, cnt=None, upto=9):
    import concourse.mybir as mybir
    import concourse.tile as tile
    from concourse import bacc
    from concourse.bass import ts

    dt = mybir.dt
    F32, BF16 = dt.float32, dt.bfloat16
    AF = mybir.ActivationFunctionType
    if cnt is None:
        cnt = (0,) + (J,) * (TK - 1)

    nc = bacc.Bacc("TRN2", target_bir_lowering=False, debug=False,
                   num_devices=NCORES)

    xT_d = nc.dram_tensor("xT", [128, CK, T], BF16, kind="ExternalInput").ap()
    xb_d = nc.dram_tensor("xb", [128, TK, C], BF16, kind="ExternalInput").ap()
    wqk_d = nc.dram_tensor("wqk", [128, CK, 2 * C], BF16, kind="ExternalInput").ap()
    wv_d = nc.dram_tensor("wv", [128, CK, C], BF16, kind="ExternalInput").ap()
    wp_d = nc.dram_tensor("wp", [128, CK, C], BF16, kind="ExternalInput").ap()
    bqk_d = nc.dram_tensor("bqk", [128, 2 * C // 128], F32, kind="ExternalInput").ap()
    beff_d = nc.dram_tensor("beff", [128, CK], F32, kind="ExternalInput").ap()
    dmask_d = nc.dram_tensor("dmask", [128, TK, 128], BF16, kind="ExternalInput").ap()
    G_d = nc.dram_tensor("G", [128, TK, J], BF16, kind="ExternalInput").ap()
    GT_d = nc.dram_tensor("GT", [J, T], BF16, kind="ExternalInput").ap()
    out_d = nc.dram_tensor("out", [128, CK, T], F32, kind="ExternalOutput").ap()

    with tile.TileContext(nc) as tc, ExitStack() as ctx:
        consts = ctx.enter_context(tc.tile_pool(name="consts", bufs=1))

        # dummy exp pulls the ACT exp-table load into the DMA-bound front
        warm = consts.tile([1, 128], F32)
        nc.gpsimd.memset(warm, 0.0)
        nc.scalar.activation(warm, warm, AF.Exp)

        # per-chunk loads so the first qk matmuls start after ~1/4 of the
        # weight+activation load; wp is only needed by the final proj.
        xT_s = consts.tile([128, CK, T], BF16)
        wqk_s = consts.tile([128, CK, 2 * C], BF16)
        for j in range(CK):
            nc.sync.dma_start(out=xT_s[:, j, :], in_=xT_d[:, j, :])
            nc.scalar.dma_start(out=wqk_s[:, j, :], in_=wqk_d[:, j, :])
        xb_s = consts.tile([128, TK, C], BF16)
        nc.sync.dma_start(out=xb_s, in_=xb_d)
        G_s = consts.tile([128, TK, J], BF16)
        nc.gpsimd.dma_start(out=G_s, in_=G_d)
        wv_s = consts.tile([128, CK, C], BF16)
        nc.scalar.dma_start(out=wv_s, in_=wv_d)
        bqk_s = consts.tile([128, 2 * C // 128], F32)
        nc.gpsimd.dma_start(out=bqk_s, in_=bqk_d)
        dmask_s = consts.tile([128, TK, 128], BF16)
        nc.gpsimd.dma_start(out=dmask_s, in_=dmask_d)
        GT_s = consts.tile([J, T], BF16)
        nc.gpsimd.dma_start(out=GT_s, in_=GT_d)
        beff_s = consts.tile([128, CK], F32)
        nc.gpsimd.dma_start(out=beff_s, in_=beff_d)
        wp_s = consts.tile([128, CK, C], BF16)
        nc.scalar.dma_start(out=wp_s, in_=wp_d)

        zrow = consts.tile([1, 512], BF16)
        nc.gpsimd.memset(zrow, 0.0)
        zcol = consts.tile([1, 128], BF16)
        nc.gpsimd.memset(zcol, 0.0)

        qkT = consts.tile([128, 2 * C // 128, T], BF16)   # tiles 0-3 q, 4-7 k
        vaug = consts.tile([128, TK, H, D + 1], BF16)
        xgT = consts.tile([128, CK, J], BF16)
        qpT = consts.tile([128, CK, J], BF16)
        ypad = consts.tile([J, 2, 4 * (D + 1)], BF16)     # raw pad num/den
        yT = consts.tile([128, CK, T], BF16)

        nc.gpsimd.memset(vaug[:, :, :, D:D + 1], 1.0)

        def run_body():
            body(nc, tc, ts, F32, BF16, AF, cnt,
                 xb_s, G_s, xT_s, wqk_s, wv_s, wp_s, bqk_s, beff_s,
                 dmask_s, GT_s, zrow, zcol, qkT, vaug, xgT, qpT, ypad, yT,
                 out_d, upto)

        if reps == 1:
            run_body()
        else:
            with tc.For_i(0, reps, 1):
                run_body()

    nc.compile()
    return nc


def body(nc, tc, ts, F32, BF16, AF, cnt,
         xb_s, G_s, xT_s, wqk_s, wv_s, wp_s, bqk_s, beff_s,
         dmask_s, GT_s, zrow, zcol, qkT, vaug, xgT, qpT, ypad, yT, out_d,
         upto=9):
    do_pad = upto >= 2
    do_attn = upto >= 3
    # ---- QKV projections + pad-row gather/QK/exp/AV, interleaved ----
    with tc.tile_pool(name="pqk", bufs=3, space="PSUM") as pqk, \
         tc.tile_pool(name="etpad", bufs=14) as etpad:

        def qk_tile(i):
            for n in range(T // 512):
                ps = pqk.tile([128, 512], F32)
                for j in range(CK):
                    nc.tensor.matmul(ps, wqk_s[:, j, ts(i, 128)],
                                     xT_s[:, j, ts(n, 512)],
                                     start=(j == 0), stop=(j == CK - 1))
                nc.vector.tensor_scalar_add(qkT[:, i, ts(n, 512)], ps,
                                            bqk_s[:, i:i + 1])

        def v_tile(i):
            ps = pqk.tile([128, 512], F32)
            for j in range(CK):
                nc.tensor.matmul(ps, xT_s[:, j, ts(i, 128)], wv_s[:, j, :],
                                 start=(j == 0), stop=(j == CK - 1))
            nc.scalar.activation(
                vaug[:, i, :, 0:D], ps.rearrange("p (h d) -> p h d", h=H),
                AF.Copy)

        # k head-tiles first: the pad pass needs them and PE can start as
        # soon as the first wqk/xT chunks land
        for i in range(4, 8):
            qk_tile(i)

        # gather padded-row x columns (xgT = x^T @ G) and project to q_pad
        # (reuses the pqk pool: fewer pools = fewer per-iteration drains)
        for ct in range(CK if do_pad else 0):
            ps = pqk.tile([128, J], F32)
            for tcp in range(TK):
                nc.tensor.matmul(ps, xb_s[:, tcp, ts(ct, 128)],
                                 G_s[:, tcp, :],
                                 start=(tcp == 0), stop=(tcp == TK - 1))
            nc.vector.tensor_copy(xgT[:, ct, :], ps)
        for pt in range(CK if do_pad else 0):
            ps = pqk.tile([128, J], F32)
            for j in range(CK):
                nc.tensor.matmul(ps, wqk_s[:, j, ts(pt, 128)],
                                 xgT[:, j, :],
                                 start=(j == 0), stop=(j == CK - 1))
            nc.vector.tensor_scalar_add(qpT[:, pt, :], ps,
                                        bqk_s[:, pt:pt + 1])

        # pad strips grouped by head parity: 64-row matmuls must keep a
        # fixed lhsT partition-base within a strip (switching the PE
        # row-quadrant between nearby 64-row matmuls aborts the NEFF), so
        # group pg covers heads h = 2*hh + pg, all at base pg*64. Strip kt
        # only computes the slot prefix [0:cnt[kt]]. q/v tiles interleave
        # as PE filler while ACT chews the pad exps.
        eps = {}
        filler = [(qk_tile, 0), (qk_tile, 1), (v_tile, 0), (v_tile, 1),
                  (qk_tile, 2), (qk_tile, 3), (v_tile, 2), (v_tile, 3),
                  (v_tile, 4), (v_tile, 5), (v_tile, 6), (v_tile, 7)]
        fi = 0
        pads = ExitStack()
        pspad = pads.enter_context(
            tc.tile_pool(name="pspad", bufs=2, space="PSUM"))
        ypadp = pads.enter_context(
            tc.tile_pool(name="ypadp", bufs=1, space="PSUM"))
        for pg_ in range(2 if do_pad else 0):
            po = pg_ * 64
            for kt in range(1, TK):
                ck = cnt[kt]
                if ck:
                    sp = pspad.tile([128, 4, J], F32)
                    for hh in range(4):
                        nc.tensor.matmul(sp[:, hh, 0:ck],
                                         qkT[po:po + D, 4 + hh, ts(kt, 128)],
                                         qpT[po:po + D, hh, 0:ck],
                                         start=(hh == 0), stop=(hh == 3),
                                         skip_group_check=True)
                    ep = etpad.tile([128, 4, J], BF16)
                    nc.scalar.activation(ep[:, :, 0:ck], sp[:, :, 0:ck],
                                         AF.Exp)
                    eps[(kt, pg_)] = ep
                if fi < len(filler):
                    f, arg = filler[fi]
                    f(arg)
                    fi += 1
        while fi < len(filler):
            f, arg = filler[fi]
            f(arg)
            fi += 1

        # pad AV: one K=1 zeros-matmul opens each half's 2KB zero-region
        # exactly once (start=True zeroes the whole bank), then everything
        # accumulates; rows >= cnt[kt] stay zero and scatter ignores them
        ypp = ypadp.tile([J, 2, 512], F32) if do_pad else None
        for pg_ in range(2 if do_pad else 0):
            nc.tensor.matmul(ypp[:, pg_, :], zcol, zrow,
                             start=True, stop=False, skip_group_check=True)
        for pg_ in range(2 if do_pad else 0):
            for kt in range(1, TK):
                ck = cnt[kt]
                if not ck:
                    continue
                ep = eps[(kt, pg_)]
                for hh in range(4):
                    h = 2 * hh + pg_
                    nc.tensor.matmul(ypp[0:ck, pg_, ts(hh, D + 1)],
                                     ep[:, hh, 0:ck], vaug[:, kt, h, :],
                                     start=False,
                                     stop=(kt == TK - 1 and hh == 3),
                                     skip_group_check=True)

        if do_pad:
            nc.vector.tensor_copy(ypad[:, 0, :], ypp[:, 0, 0:4 * (D + 1)])
            nc.vector.tensor_copy(ypad[:, 1, :], ypp[:, 1, 0:4 * (D + 1)])
        pads.close()

    # ---- causal attention, head-outer, software-pipelined across heads ----
    with tc.tile_pool(name="ps_s", bufs=2, space="PSUM") as ps_s, \
         tc.tile_pool(name="ps_y", bufs=2, space="PSUM") as ps_y, \
         tc.tile_pool(name="expp", bufs=14) as expp, \
         tc.tile_pool(name="rnp", bufs=6) as rnp:
        ets = {}

        def emit_qk(h):
            # k-tile strips; the late small strips share one PSUM tile and
            # one exp (single accumulation group per bank; mid-bank output
            # offsets are fine)
            hp, po = h // 2, (h % 2) * 64
            sps = {}
            for kt in range(TK):
                key, poff = _PAIR[kt]
                if kt == key:
                    sp = ps_s.tile([128, T], F32)
                    sps[key] = sp
                else:
                    sp = sps[key]
                q0 = 128 * kt
                w = T - q0
                for off, n in _chunks(w):
                    o = poff + off
                    nc.tensor.matmul(sp[:, o:o + n],
                                     qkT[po:po + D, 4 + hp, ts(kt, 128)],
                                     qkT[po:po + D, hp, q0 + off:q0 + off + n],
                                     start=(o % 512 == 0), stop=(kt == key + 1
                                                                 or kt <= 3),
                                     skip_group_check=True)
                if kt == key + 1 or kt <= 3:
                    et = expp.tile([128, T], BF16)
                    wtot = poff + w
                    nc.scalar.activation(et[:, 0:wtot], sp[:, 0:wtot], AF.Exp)
                    ets[(h, key)] = et
            for kt in range(TK):
                key, poff = _PAIR[kt]
                nc.vector.tensor_mul(ets[(h, key)][:, poff:poff + 128],
                                     ets[(h, key)][:, poff:poff + 128],
                                     dmask_s[:, kt, :])

        def emit_av(h):
            hp, po = h // 2, (h % 2) * 64
            half, hh = h % 2, h // 2
            yp = ps_y.tile([D + 1, T], F32)
            for kt in range(TK):
                key, poff = _PAIR[kt]
                et = ets[(h, key)]
                q0 = 128 * kt
                a = q0
                for bnd in (512, T):
                    if a < bnd:
                        n = bnd - a
                        nc.tensor.matmul(
                            yp[:, a:a + n], vaug[:, kt, h, :],
                            et[:, poff + a - q0:poff + a - q0 + n],
                            start=(kt == 0), stop=False,
                            skip_group_check=True)
                        a = bnd
            # scatter pad-row num/den contributions into this head's PSUM
            for off, n in _chunks(T):
                nc.tensor.matmul(yp[:, off:off + n],
                                 ypad[:, half, ts(hh, D + 1)],
                                 GT_s[:, off:off + n],
                                 start=False, stop=True,
                                 skip_group_check=True)
            # normalize in q-halves: pipelines recip/broadcast/mul and lets
            # the projection start on the first half of yT sooner
            for hq in (0, 512):
                rec = rnp.tile([1, 512], F32)
                nc.vector.reciprocal(rec, yp[D:D + 1, hq:hq + 512])
                rb = rnp.tile([D, 512], F32)
                nc.gpsimd.partition_broadcast(rb, rec)
                nc.vector.tensor_mul(yT[po:po + D, hp, hq:hq + 512],
                                     yp[0:D, hq:hq + 512], rb)

        for h in range(H if do_attn else 0):
            emit_qk(h)
            if h >= 1:
                emit_av(h - 1)
        if do_attn:
            emit_av(H - 1)

    # ---- out^T = W_p^T y^T + b_eff, stored transposed ----
    with tc.tile_pool(name="pp", bufs=4, space="PSUM") as pp, \
         tc.tile_pool(name="outst", bufs=4) as outst:
        for qt in range(TK):
            op = pp.tile([128, CK, 128], F32)
            for ct in range(CK):
                for cin in range(CK):
                    nc.tensor.matmul(op[:, ct, :],
                                     wp_s[:, cin, ts(ct, 128)],
                                     yT[:, cin, ts(qt, 128)],
                                     start=(ct == 0 and cin == 0),
                                     stop=(ct == CK - 1 and cin == CK - 1),
                                     skip_group_check=True)
            ot = outst.tile([128, CK, 128], F32)
            for ct in range(CK):
                if ct % 2 == 0:
                    nc.scalar.activation(ot[:, ct, :], op[:, ct, :],
                                         AF.Identity,
                                         bias=beff_s[:, ct:ct + 1])
                else:
                    nc.vector.tensor_scalar_add(ot[:, ct, :], op[:, ct, :],
                                                beff_s[:, ct:ct + 1])
            nc.sync.dma_start(out=out_d[:, :, ts(qt, 128)], in_=ot)


def get_nc(reps=1, upto=9):
    cnt = _CNT["profile"]
    key = ("nc", reps, cnt, upto)
    if key not in _CACHE:
        _CACHE[key] = _build_nc(reps, cnt, upto)
    return _CACHE[key]


def _slot_layout(padding_mask):
    """Group padded-row slots by query tile with capacities shared across
    all TT groups so one NEFF serves every core; returns the layout."""
    counts = np.zeros((TT, TK), np.int64)
    for tt in range(TT):
        qt = np.where(padding_mask[tt])[0] // 128
        for g in range(TK):
            counts[tt, g] = (qt == g).sum()
    caps = counts.max(0)
    assert caps.sum() <= J, f"slot profile {caps.sum()} exceeds J={J}"
    slot0 = np.concatenate([[0], np.cumsum(caps)])
    cnt = tuple(int(slot0[kt]) for kt in range(TK))  # slots with group < kt
    return slot0, cnt


def make_in_maps(x, padding_mask, W_qkv, b_qkv, W_proj, b_proj):
    BF = ml_dtypes.bfloat16
    x = np.asarray(x, np.float32)
    padding_mask = np.asarray(padding_mask, bool)
    W_qkv = np.asarray(W_qkv, np.float32)
    b_qkv = np.asarray(b_qkv, np.float32)
    W_proj = np.asarray(W_proj, np.float32)
    b_proj = np.asarray(b_proj, np.float32)

    scale = np.float32(1.0 / np.sqrt(D))
    wqk = np.concatenate([W_qkv[:, :C] * scale, W_qkv[:, C:2 * C]], axis=1)
    wqk = np.ascontiguousarray(
        wqk.reshape(CK, 128, 2 * C).transpose(1, 0, 2)).astype(BF)
    wv = np.ascontiguousarray(
        W_qkv[:, 2 * C:].reshape(CK, 128, C).transpose(1, 0, 2)).astype(BF)
    wp = np.ascontiguousarray(
        W_proj.reshape(CK, 128, C).transpose(1, 0, 2)).astype(BF)
    bqk = np.concatenate([b_qkv[:C] * scale, b_qkv[C:2 * C]])
    bqk = np.ascontiguousarray(bqk.reshape(-1, 128).T).astype(np.float32)
    beff = (b_qkv[2 * C:] @ W_proj + b_proj)
    beff = np.ascontiguousarray(beff.reshape(CK, 128).T).astype(np.float32)

    slot0, cnt = _slot_layout(padding_mask)
    _CNT["profile"] = cnt

    pp = np.arange(128)
    in_maps = []
    for b in range(B):
        tt = b % TT
        pad = padding_mask[tt]
        idx = np.where(pad)[0]
        # slot assignment: query-tile-g rows occupy slots [slot0[g], ...)
        slots = np.empty(len(idx), np.int64)
        for g in range(TK):
            sel = (idx // 128) == g
            slots[sel] = slot0[g] + np.arange(sel.sum())

        G = np.zeros((T, J), np.float32)
        G[idx, slots] = 1.0
        G = np.ascontiguousarray(
            G.reshape(TK, 128, J).transpose(1, 0, 2)).astype(BF)
        GT = np.zeros((J, T), np.float32)
        GT[slots, idx] = 1.0
        GT = GT.astype(BF)
        # dmask[p, kt, q] = (p <= q) | pad[128*kt + q]
        dmask = (pp[:, None, None] <= pp[None, None, :]) | \
            pad.reshape(TK, 128)[None, :, :]
        dmask = np.ascontiguousarray(dmask).astype(BF)

        xb_full = x[b]
        xT = np.ascontiguousarray(
            xb_full.T.reshape(CK, 128, T).transpose(1, 0, 2)).astype(BF)
        xb = np.ascontiguousarray(
            xb_full.reshape(TK, 128, C).transpose(1, 0, 2)).astype(BF)

        in_maps.append({
            "xT": xT, "xb": xb, "wqk": wqk, "wv": wv, "wp": wp,
            "bqk": bqk, "beff": beff, "dmask": dmask,
            "G": G, "GT": GT,
        })
    return in_maps


def kernel(x, padding_mask, W_qkv, b_qkv, W_proj, b_proj):
    from concourse.bass_utils import run_bass_kernel_spmd

    in_maps = make_in_maps(x, padding_mask, W_qkv, b_qkv, W_proj, b_proj)
    nc = get_nc()
    res = run_bass_kernel_spmd(nc, in_maps, list(range(NCORES)))
    outs = []
    for b in range(B):
        a = res.results[b]["out"]          # [128, CK, T] = out^T tiled
        outT = a.transpose(1, 0, 2).reshape(C, T)
        outs.append(outT.T)
    return np.ascontiguousarray(np.stack(outs)).astype(np.float32)


# revision 36
# speedup vs baseline: 1.1707x; 1.1406x over previous
"""Fused multi-head causal+padding attention for Trainium2 (Bass/Tile).

Problem: nn_Attention (B=8, T=1024, C=512, H=8, D=64, TT=4), f32.
Sharding: data-parallel over batch B across 8 NeuronCores (1 batch elem/core).

v2 design (all-bf16 matmuls, causal block-skip + padded-row fix-up):
  mask[q,k] = (k <= q) | pad[q]  -> 90% of rows are causal, 10% attend to all k.
  - QKV: qk^T = W_qk^T @ x^T (heads in [d,t] layout), v = x @ W_v ([t,d]).
    x arrives pre-transposed from host (both x and x^T in bf16): no on-chip
    transposes. DMAs are split per-contraction-chunk so the first matmuls
    start after ~1/4 of the weight load.
  - Causal pass (per head, k-tile strips): S^T[k, q>=128*kt] only (skips
    fully-masked above-diagonal blocks: 36/64 remain), one exp per strip
    (late small strips paired to amortize ACT PSUM-access init), bf16
    multiplicative mask on the diagonal 128-block only, AV accumulates
    y[65, q] with an appended ones column as the softmax denominator.
    Heads are software-pipelined: QK strips of head h+1 are emitted before
    the AVs of head h so PE fills the exp latency.
  - Pad pass: padded-row queries are gathered via a host-built one-hot
    matrix G (gather = matmul), projected to q_pad, attended over exactly
    the k-tiles the causal pass skipped, and the raw numerator/denominator
    contributions are scattered back into each head's PSUM accumulator
    with G^T (scatter = matmul, accumulate). Slots are grouped by query
    tile (same slot->tile profile on every core, dummy zero-columns pad
    the groups), so k-tile kt only processes the slot prefix cnt[kt] and
    no pad-side masking is needed at all.
  - Normalize via reciprocal + partition_broadcast, then out^T = W_p^T y^T
    (+bias) stored transposed; host untransposes.

HW gotchas encoded here: 64-row matmuls must keep a fixed lhsT partition
base between nearby instructions (PE row-quadrant switching aborts the
NEFF), PSUM matmul start=True zeroes the whole 2KB bank (one accumulation
group per bank; K=1 zeros-matmul opens shared banks), GPSIMD cannot touch
PSUM. All matmuls bf16 (1 cycle/row at any N).
"""

import numpy as np
import ml_dtypes
from contextlib import ExitStack

B, T, C, H, TT = 8, 1024, 512, 8, 4
D = C // H
NCORES = 8
TK = T // 128   # 8 t-tiles
CK = C // 128   # 4 c-tiles
J = 128         # gathered padded-row slot capacity (profile sum <= 128)

_CACHE = {}
_CNT = {"profile": None}   # slot-prefix profile, set by make_in_maps

# exp strip pairing: kt -> (tile key, column offset inside the tile)
_PAIR = {0: (0, 0), 1: (1, 0), 2: (2, 0), 3: (3, 0),
         4: (4, 0), 5: (4, 512), 6: (6, 0), 7: (6, 256)}


def _chunks(w, step=512):
    off = 0
    while off < w:
        n = min(step, w - off)
        yield off, n
        off += n


def _build_nc(reps=1, cnt=None, upto=9, twin=False):
    import concourse.mybir as mybir
    import concourse.tile as tile
    from concourse import bacc
    from concourse.bass import ts

    dt = mybir.dt
    F32, BF16 = dt.float32, dt.bfloat16
    AF = mybir.ActivationFunctionType
    if cnt is None:
        cnt = (0,) + (J,) * (TK - 1)

    nc = bacc.Bacc("TRN2", target_bir_lowering=False, debug=False,
                   num_devices=NCORES)

    xT_d = nc.dram_tensor("xT", [128, CK, T], BF16, kind="ExternalInput").ap()
    xb_d = nc.dram_tensor("xb", [128, TK, C], BF16, kind="ExternalInput").ap()
    wqk_d = nc.dram_tensor("wqk", [128, CK, 2 * C], BF16, kind="ExternalInput").ap()
    wv_d = nc.dram_tensor("wv", [128, CK, C], BF16, kind="ExternalInput").ap()
    wp_d = nc.dram_tensor("wp", [128, CK, C], BF16, kind="ExternalInput").ap()
    bqk_d = nc.dram_tensor("bqk", [128, 2 * C // 128], F32, kind="ExternalInput").ap()
    beff_d = nc.dram_tensor("beff", [128, CK], F32, kind="ExternalInput").ap()
    dmask_d = nc.dram_tensor("dmask", [128, TK, 128], BF16, kind="ExternalInput").ap()
    G_d = nc.dram_tensor("G", [128, TK, J], BF16, kind="ExternalInput").ap()
    GT_d = nc.dram_tensor("GT", [J, T], BF16, kind="ExternalInput").ap()
    out_d = nc.dram_tensor("out", [128, CK, T], F32, kind="ExternalOutput").ap()

    with tile.TileContext(nc) as tc, ExitStack() as ctx:
        consts = ctx.enter_context(tc.tile_pool(name="consts", bufs=1))

        # dummy exp pulls the ACT exp-table load into the DMA-bound front
        warm = consts.tile([1, 128], F32)
        nc.gpsimd.memset(warm, 0.0)
        nc.scalar.activation(warm, warm, AF.Exp)

        # per-chunk loads so the first qk matmuls start after ~1/4 of the
        # weight+activation load; wp is only needed by the final proj.
        xT_s = consts.tile([128, CK, T], BF16)
        wqk_s = consts.tile([128, CK, 2 * C], BF16)
        for j in range(CK):
            nc.sync.dma_start(out=xT_s[:, j, :], in_=xT_d[:, j, :])
            nc.scalar.dma_start(out=wqk_s[:, j, :], in_=wqk_d[:, j, :])
        xb_s = consts.tile([128, TK, C], BF16)
        nc.sync.dma_start(out=xb_s, in_=xb_d)
        G_s = consts.tile([128, TK, J], BF16)
        nc.gpsimd.dma_start(out=G_s, in_=G_d)
        wv_s = consts.tile([128, CK, C], BF16)
        nc.scalar.dma_start(out=wv_s, in_=wv_d)
        bqk_s = consts.tile([128, 2 * C // 128], F32)
        nc.gpsimd.dma_start(out=bqk_s, in_=bqk_d)
        dmask_s = consts.tile([128, TK, 128], BF16)
        nc.gpsimd.dma_start(out=dmask_s, in_=dmask_d)
        GT_s = consts.tile([J, T], BF16)
        nc.gpsimd.dma_start(out=GT_s, in_=GT_d)
        beff_s = consts.tile([128, CK], F32)
        nc.gpsimd.dma_start(out=beff_s, in_=beff_d)
        wp_s = consts.tile([128, CK, C], BF16)
        nc.scalar.dma_start(out=wp_s, in_=wp_d)

        zrow = consts.tile([1, 512], BF16)
        nc.gpsimd.memset(zrow, 0.0)
        zcol = consts.tile([1, 128], BF16)
        nc.gpsimd.memset(zcol, 0.0)

        sets = []
        for si in range(2 if twin else 1):
            qkT = consts.tile([128, 2 * C // 128, T], BF16, name=f"qkT{si}")
            vaug = consts.tile([128, TK, H, D + 1], BF16, name=f"vaug{si}")
            xgT = consts.tile([128, CK, J], BF16, name=f"xgT{si}")
            qpT = consts.tile([128, CK, J], BF16, name=f"qpT{si}")
            ypad = consts.tile([J, 2, 4 * (D + 1)], BF16, name=f"ypad{si}")
            yT = consts.tile([128, CK, T], BF16, name=f"yT{si}")
            nc.gpsimd.memset(vaug[:, :, :, D:D + 1], 1.0)
            sets.append((qkT, vaug, xgT, qpT, ypad, yT))

        def run_body(si=0):
            qkT, vaug, xgT, qpT, ypad, yT = sets[si]
            body(nc, tc, ts, F32, BF16, AF, cnt,
                 xb_s, G_s, xT_s, wqk_s, wv_s, wp_s, bqk_s, beff_s,
                 dmask_s, GT_s, zrow, zcol, qkT, vaug, xgT, qpT, ypad, yT,
                 out_d, upto)

        if reps == 1:
            run_body()
            if twin:
                run_body(1)
        else:
            with tc.For_i(0, reps, 1):
                run_body()
                if twin:
                    run_body(1)

    nc.compile()
    return nc


def body(nc, tc, ts, F32, BF16, AF, cnt,
         xb_s, G_s, xT_s, wqk_s, wv_s, wp_s, bqk_s, beff_s,
         dmask_s, GT_s, zrow, zcol, qkT, vaug, xgT, qpT, ypad, yT, out_d,
         upto=9):
    do_pad = upto >= 2
    do_attn = upto >= 3
    # ---- QKV projections + pad-row gather/QK/exp/AV, interleaved ----
    with tc.tile_pool(name="pqk", bufs=3, space="PSUM") as pqk, \
         tc.tile_pool(name="etpad", bufs=14) as etpad:

        def qk_tile(i):
            for n in range(T // 512):
                ps = pqk.tile([128, 512], F32)
                for j in range(CK):
                    nc.tensor.matmul(ps, wqk_s[:, j, ts(i, 128)],
                                     xT_s[:, j, ts(n, 512)],
                                     start=(j == 0), stop=(j == CK - 1))
                nc.vector.tensor_scalar_add(qkT[:, i, ts(n, 512)], ps,
                                            bqk_s[:, i:i + 1])

        def v_tile(i):
            ps = pqk.tile([128, 512], F32)
            for j in range(CK):
                nc.tensor.matmul(ps, xT_s[:, j, ts(i, 128)], wv_s[:, j, :],
                                 start=(j == 0), stop=(j == CK - 1))
            nc.scalar.activation(
                vaug[:, i, :, 0:D], ps.rearrange("p (h d) -> p h d", h=H),
                AF.Copy)

        # k head-tiles first: the pad pass needs them and PE can start as
        # soon as the first wqk/xT chunks land
        for i in range(4, 8):
            qk_tile(i)

        # gather padded-row x columns (xgT = x^T @ G) and project to q_pad
        # (reuses the pqk pool: fewer pools = fewer per-iteration drains)
        for ct in range(CK if do_pad else 0):
            ps = pqk.tile([128, J], F32)
            for tcp in range(TK):
                nc.tensor.matmul(ps, xb_s[:, tcp, ts(ct, 128)],
                                 G_s[:, tcp, :],
                                 start=(tcp == 0), stop=(tcp == TK - 1))
            nc.vector.tensor_copy(xgT[:, ct, :], ps)
        for pt in range(CK if do_pad else 0):
            ps = pqk.tile([128, J], F32)
            for j in range(CK):
                nc.tensor.matmul(ps, wqk_s[:, j, ts(pt, 128)],
                                 xgT[:, j, :],
                                 start=(j == 0), stop=(j == CK - 1))
            nc.vector.tensor_scalar_add(qpT[:, pt, :], ps,
                                        bqk_s[:, pt:pt + 1])

        # pad strips grouped by head parity: 64-row matmuls must keep a
        # fixed lhsT partition-base within a strip (switching the PE
        # row-quadrant between nearby 64-row matmuls aborts the NEFF), so
        # group pg covers heads h = 2*hh + pg, all at base pg*64. Strip kt
        # only computes the slot prefix [0:cnt[kt]]. q/v tiles interleave
        # as PE filler while ACT chews the pad exps.
        eps = {}
        filler = [(qk_tile, 0), (qk_tile, 1), (v_tile, 0), (v_tile, 1),
                  (qk_tile, 2), (qk_tile, 3), (v_tile, 2), (v_tile, 3),
                  (v_tile, 4), (v_tile, 5), (v_tile, 6), (v_tile, 7)]
        fi = 0
        pads = ExitStack()
        pspad = pads.enter_context(
            tc.tile_pool(name="pspad", bufs=2, space="PSUM"))
        ypadp = pads.enter_context(
            tc.tile_pool(name="ypadp", bufs=1, space="PSUM"))
        for pg_ in range(2 if do_pad else 0):
            po = pg_ * 64
            for kt in range(1, TK):
                ck = cnt[kt]
                if ck:
                    sp = pspad.tile([128, 4, J], F32)
                    for hh in range(4):
                        nc.tensor.matmul(sp[:, hh, 0:ck],
                                         qkT[po:po + D, 4 + hh, ts(kt, 128)],
                                         qpT[po:po + D, hh, 0:ck],
                                         start=(hh == 0), stop=(hh == 3),
                                         skip_group_check=True)
                    ep = etpad.tile([128, 4, J], BF16)
                    nc.scalar.activation(ep[:, :, 0:ck], sp[:, :, 0:ck],
                                         AF.Exp)
                    eps[(kt, pg_)] = ep
                if fi < len(filler):
                    f, arg = filler[fi]
                    f(arg)
                    fi += 1
        while fi < len(filler):
            f, arg = filler[fi]
            f(arg)
            fi += 1

        # pad AV: one K=1 zeros-matmul opens each half's 2KB zero-region
        # exactly once (start=True zeroes the whole bank), then everything
        # accumulates; rows >= cnt[kt] stay zero and scatter ignores them
        ypp = None
        if do_pad:
            ypp = ypadp.tile([J, 2, 512], F32, name="ypp")
        for pg_ in range(2 if do_pad else 0):
            nc.tensor.matmul(ypp[:, pg_, :], zcol, zrow,
                             start=True, stop=False, skip_group_check=True)
        for pg_ in range(2 if do_pad else 0):
            for kt in range(1, TK):
                ck = cnt[kt]
                if not ck:
                    continue
                ep = eps[(kt, pg_)]
                for hh in range(4):
                    h = 2 * hh + pg_
                    nc.tensor.matmul(ypp[0:ck, pg_, ts(hh, D + 1)],
                                     ep[:, hh, 0:ck], vaug[:, kt, h, :],
                                     start=False,
                                     stop=(kt == TK - 1 and hh == 3),
                                     skip_group_check=True)

        if do_pad:
            nc.vector.tensor_copy(ypad[:, 0, :], ypp[:, 0, 0:4 * (D + 1)])
            nc.vector.tensor_copy(ypad[:, 1, :], ypp[:, 1, 0:4 * (D + 1)])
        pads.close()

    # ---- causal attention, head-outer, software-pipelined across heads ----
    with tc.tile_pool(name="ps_s", bufs=2, space="PSUM") as ps_s, \
         tc.tile_pool(name="ps_y", bufs=2, space="PSUM") as ps_y, \
         tc.tile_pool(name="expp", bufs=14) as expp, \
         tc.tile_pool(name="rnp", bufs=6) as rnp:
        ets = {}

        def emit_qk(h):
            # k-tile strips; the late small strips share one PSUM tile and
            # one exp (single accumulation group per bank; mid-bank output
            # offsets are fine)
            hp, po = h // 2, (h % 2) * 64
            sps = {}
            for kt in range(TK):
                key, poff = _PAIR[kt]
                if kt == key:
                    sp = ps_s.tile([128, T], F32)
                    sps[key] = sp
                else:
                    sp = sps[key]
                q0 = 128 * kt
                w = T - q0
                for off, n in _chunks(w):
                    o = poff + off
                    nc.tensor.matmul(sp[:, o:o + n],
                                     qkT[po:po + D, 4 + hp, ts(kt, 128)],
                                     qkT[po:po + D, hp, q0 + off:q0 + off + n],
                                     start=(o % 512 == 0), stop=(kt == key + 1
                                                                 or kt <= 3),
                                     skip_group_check=True)
                if kt == key + 1 or kt <= 3:
                    et = expp.tile([128, T], BF16)
                    wtot = poff + w
                    nc.scalar.activation(et[:, 0:wtot], sp[:, 0:wtot], AF.Exp)
                    ets[(h, key)] = et
            for kt in range(TK):
                key, poff = _PAIR[kt]
                nc.vector.tensor_mul(ets[(h, key)][:, poff:poff + 128],
                                     ets[(h, key)][:, poff:poff + 128],
                                     dmask_s[:, kt, :])

        def emit_av(h):
            hp, po = h // 2, (h % 2) * 64
            half, hh = h % 2, h // 2
            yp = ps_y.tile([D + 1, T], F32)
            for kt in range(TK):
                key, poff = _PAIR[kt]
                et = ets[(h, key)]
                q0 = 128 * kt
                a = q0
                for bnd in (512, T):
                    if a < bnd:
                        n = bnd - a
                        nc.tensor.matmul(
                            yp[:, a:a + n], vaug[:, kt, h, :],
                            et[:, poff + a - q0:poff + a - q0 + n],
                            start=(kt == 0), stop=False,
                            skip_group_check=True)
                        a = bnd
            # scatter pad-row num/den contributions into this head's PSUM
            for off, n in _chunks(T):
                nc.tensor.matmul(yp[:, off:off + n],
                                 ypad[:, half, ts(hh, D + 1)],
                                 GT_s[:, off:off + n],
                                 start=False, stop=True,
                                 skip_group_check=True)
            # normalize in q-halves: pipelines recip/broadcast/mul and lets
            # the projection start on the first half of yT sooner
            for hq in (0, 512):
                rec = rnp.tile([1, 512], F32)
                nc.vector.reciprocal(rec, yp[D:D + 1, hq:hq + 512])
                rb = rnp.tile([D, 512], F32)
                nc.gpsimd.partition_broadcast(rb, rec)
                nc.vector.tensor_mul(yT[po:po + D, hp, hq:hq + 512],
                                     yp[0:D, hq:hq + 512], rb)

        for h in range(H if do_attn else 0):
            emit_qk(h)
            if h >= 1:
                emit_av(h - 1)
        if do_attn:
            emit_av(H - 1)

    # ---- out^T = W_p^T y^T + b_eff, stored transposed ----
    with tc.tile_pool(name="pp", bufs=4, space="PSUM") as pp, \
         tc.tile_pool(name="outst", bufs=4) as outst:
        for qt in range(TK):
            op = pp.tile([128, CK, 128], F32)
            for ct in range(CK):
                for cin in range(CK):
                    nc.tensor.matmul(op[:, ct, :],
                                     wp_s[:, cin, ts(ct, 128)],
                                     yT[:, cin, ts(qt, 128)],
                                     start=(ct == 0 and cin == 0),
                                     stop=(ct == CK - 1 and cin == CK - 1),
                                     skip_group_check=True)
            ot = outst.tile([128, CK, 128], F32)
            for ct in range(CK):
                if ct % 2 == 0:
                    nc.scalar.activation(ot[:, ct, :], op[:, ct, :],
                                         AF.Identity,
                                         bias=beff_s[:, ct:ct + 1])
                else:
                    nc.vector.tensor_scalar_add(ot[:, ct, :], op[:, ct, :],
                                                beff_s[:, ct:ct + 1])
            nc.sync.dma_start(out=out_d[:, :, ts(qt, 128)], in_=ot)


def get_nc(reps=1, upto=9, twin=False):
    cnt = _CNT["profile"]
    key = ("nc", reps, cnt, upto, twin)
    if key not in _CACHE:
        _CACHE[key] = _build_nc(reps, cnt, upto, twin)
    return _CACHE[key]


def _slot_layout(padding_mask):
    """Group padded-row slots by query tile with capacities shared across
    all TT groups so one NEFF serves every core; returns the layout."""
    counts = np.zeros((TT, TK), np.int64)
    for tt in range(TT):
        qt = np.where(padding_mask[tt])[0] // 128
        for g in range(TK):
            counts[tt, g] = (qt == g).sum()
    caps = counts.max(0)
    assert caps.sum() <= J, f"slot profile {caps.sum()} exceeds J={J}"
    slot0 = np.concatenate([[0], np.cumsum(caps)])
    cnt = tuple(int(slot0[kt]) for kt in range(TK))  # slots with group < kt
    return slot0, cnt


def make_in_maps(x, padding_mask, W_qkv, b_qkv, W_proj, b_proj):
    BF = ml_dtypes.bfloat16
    x = np.asarray(x, np.float32)
    padding_mask = np.asarray(padding_mask, bool)
    W_qkv = np.asarray(W_qkv, np.float32)
    b_qkv = np.asarray(b_qkv, np.float32)
    W_proj = np.asarray(W_proj, np.float32)
    b_proj = np.asarray(b_proj, np.float32)

    scale = np.float32(1.0 / np.sqrt(D))
    wqk = np.concatenate([W_qkv[:, :C] * scale, W_qkv[:, C:2 * C]], axis=1)
    wqk = np.ascontiguousarray(
        wqk.reshape(CK, 128, 2 * C).transpose(1, 0, 2)).astype(BF)
    wv = np.ascontiguousarray(
        W_qkv[:, 2 * C:].reshape(CK, 128, C).transpose(1, 0, 2)).astype(BF)
    wp = np.ascontiguousarray(
        W_proj.reshape(CK, 128, C).transpose(1, 0, 2)).astype(BF)
    bqk = np.concatenate([b_qkv[:C] * scale, b_qkv[C:2 * C]])
    bqk = np.ascontiguousarray(bqk.reshape(-1, 128).T).astype(np.float32)
    beff = (b_qkv[2 * C:] @ W_proj + b_proj)
    beff = np.ascontiguousarray(beff.reshape(CK, 128).T).astype(np.float32)

    slot0, cnt = _slot_layout(padding_mask)
    _CNT["profile"] = cnt

    pp = np.arange(128)
    in_maps = []
    for b in range(B):
        tt = b % TT
        pad = padding_mask[tt]
        idx = np.where(pad)[0]
        # slot assignment: query-tile-g rows occupy slots [slot0[g], ...)
        slots = np.empty(len(idx), np.int64)
        for g in range(TK):
            sel = (idx // 128) == g
            slots[sel] = slot0[g] + np.arange(sel.sum())

        G = np.zeros((T, J), np.float32)
        G[idx, slots] = 1.0
        G = np.ascontiguousarray(
            G.reshape(TK, 128, J).transpose(1, 0, 2)).astype(BF)
        GT = np.zeros((J, T), np.float32)
        GT[slots, idx] = 1.0
        GT = GT.astype(BF)
        # dmask[p, kt, q] = (p <= q) | pad[128*kt + q]
        dmask = (pp[:, None, None] <= pp[None, None, :]) | \
            pad.reshape(TK, 128)[None, :, :]
        dmask = np.ascontiguousarray(dmask).astype(BF)

        xb_full = x[b]
        xT = np.ascontiguousarray(
            xb_full.T.reshape(CK, 128, T).transpose(1, 0, 2)).astype(BF)
        xb = np.ascontiguousarray(
            xb_full.reshape(TK, 128, C).transpose(1, 0, 2)).astype(BF)

        in_maps.append({
            "xT": xT, "xb": xb, "wqk": wqk, "wv": wv, "wp": wp,
            "bqk": bqk, "beff": beff, "dmask": dmask,
            "G": G, "GT": GT,
        })
    return in_maps


def kernel(x, padding_mask, W_qkv, b_qkv, W_proj, b_proj):
    from concourse.bass_utils import run_bass_kernel_spmd

    in_maps = make_in_maps(x, padding_mask, W_qkv, b_qkv, W_proj, b_proj)
    nc = get_nc()
    res = run_bass_kernel_spmd(nc, in_maps, list(range(NCORES)))
    outs = []
    for b in range(B):
        a = res.results[b]["out"]          # [128, CK, T] = out^T tiled
        outT = a.transpose(1, 0, 2).reshape(C, T)
        outs.append(outT.T)
    return np.ascontiguousarray(np.stack(outs)).astype(np.float32)
